# revision 19
# baseline (speedup 1.0000x reference)
"""Expert-choice MoE layer (NucleusMoELayer) on 8 Trainium2 NeuronCores.

Strategy (expert-parallel):
 - one expert per core; router + gate-normalization replicated from an
   AllGathered logit table; shared expert sharded over tokens (1024/core)
 - router logits for the core's own 1024-token shard via fp32 PE matmul,
   AllGather -> every core sees all [8 experts x 8192 tokens] logits
 - expert-choice top-1024-per-(batch,expert) via 32-step threshold bisection
   on logits (sigmoid is monotone so logit ranking == score ranking)
 - compaction (selected tokens -> dense slots) via per-partition cumsum +
   GpSimd local_scatter of (token-id, gate-hi, gate-lo) uint16 payloads
 - dispatch: indirect-DMA row gather of selected tokens, PE-transpose to
   feature-major, swiglu FFN in bf16 (weights streamed + cast on the fly)
 - outputs are feature-major; host transposes, concatenates shared shards and
   scatter-adds routed outputs (unique indices per core)

kernel(**inputs) takes FULL unsharded inputs, returns the FULL output.
"""

import sys

if "/opt/trn_rl_repo" not in sys.path:
    sys.path.insert(0, "/opt/trn_rl_repo")

import numpy as np

import concourse.bacc as bacc
import concourse.bass as bass
import concourse.mybir as mybir
import concourse.tile as tile
from concourse.bass_utils import run_bass_kernel_spmd

dt = mybir.dt
AF = mybir.ActivationFunctionType
ALU = mybir.AluOpType

NCORES = 8
BS, SLEN, DIM = 2, 4096, 1024
INNER = 2048
I2 = 2 * INNER  # 4096
E = 8
CAP = 1024  # tokens per (batch, expert)
T = BS * SLEN  # 8192 global tokens
TSH = T // NCORES  # 1024-token shard per core
SLOTS = BS * CAP  # 2048 routed slots per expert
KD = DIM // 128  # 8 k-chunks over dim
MI = I2 // 128  # 32 m-chunks over 2*inner
KI = INNER // 128  # 16 k-chunks over inner
BISECT_ITERS = 32


def build_nc():
    nc = bacc.Bacc(None, target_bir_lowering=False, num_devices=NCORES)

    tens = {}

    def din(name, shape, dtype=dt.float32):
        tens[name] = nc.dram_tensor(name, shape, dtype, kind="ExternalInput")

    def dout(name, shape, dtype=dt.float32):
        tens[name] = nc.dram_tensor(name, shape, dtype, kind="ExternalOutput")

    din("hs", [T, DIM])
    din("hs_sh", [TSH, DIM])
    din("hsu_sh", [TSH, DIM])
    din("ts", [BS, DIM])
    din("wg", [E, 2 * DIM])
    din("w1", [DIM, I2])
    din("w2", [INNER, DIM])
    din("ws1", [DIM, I2])
    din("ws2", [INNER, DIM])
    din("esel", [128, 16])
    din("bsel", [BS, 1])
    din("b2", [16, BS])
    din("lt16", [16, 16])
    din("rsmat", [128, 128])
    din("dsmat", [128, 128])
    din("ident", [128, 128])
    din("ones128", [1, 128])
    din("iota_tid", [16, 512], dt.uint16)
    din("iota_sh", [128, 64], dt.int16)
    dout("out_routed", [DIM, SLOTS])
    dout("out_idx", [SLOTS, 1], dt.int32)
    dout("out_shared", [DIM, TSH])
    dout("dbg_thr", [128, 1])
    dout("dbg_gate", [BS, CAP])

    with tile.TileContext(nc, num_cores=NCORES) as tc:
        _emit(nc, tc, tens)
    nc.finalize()
    return nc


def _load_weights_bf16(nc, pools, wdram, nk, width, tagp):
    """Load [nk*128, width] f32 DRAM weights into resident bf16 k-tiles."""
    stg_pool, wr_pool = pools["sb"], pools["wres"]
    tiles = []
    for k in range(nk):
        wt = wr_pool.tile([128, width], dt.bfloat16, tag=f"{tagp}{k}")
        for q in range(0, width, 1024):
            stg = stg_pool.tile([128, 1024], dt.float32, tag="row_in")
            nc.sync.dma_start(
                stg[:], wdram[k * 128 : (k + 1) * 128, q : q + 1024]
            )
            nc.scalar.activation(wt[:, q : q + 1024], stg[:], AF.Copy)
        tiles.append(wt)
    return tiles


def _ffn_chunk(nc, pools, wr1, wr2, xT, rhs_off, out_dram, out_col, gate_sb):
    """One 512-token swiglu FFN chunk, feature-major, resident bf16 weights.
    wr1: 8 k-tiles [128, I2]; wr2: 16 k-tiles [128, DIM]; xT [128, KD*xw]."""
    sb, sb1, pmm = pools["sb"], pools["sb1"], pools["pmm"]
    xw = xT.shape[1] // KD
    h_sb = sb1.tile([128, 16 * 512], dt.bfloat16, tag="h_sb")
    for i in range(16):
        ps_a = pmm.tile([128, 512], dt.float32, tag="mm")
        for k in range(KD):
            nc.tensor.matmul(
                ps_a[:],
                lhsT=wr1[k][:, i * 128 : (i + 1) * 128],
                rhs=xT[:, k * xw + rhs_off : k * xw + rhs_off + 512],
                start=(k == 0),
                stop=(k == KD - 1),
            )
        ps_g = pmm.tile([128, 512], dt.float32, tag="mm")
        for k in range(KD):
            nc.tensor.matmul(
                ps_g[:],
                lhsT=wr1[k][:, (16 + i) * 128 : (17 + i) * 128],
                rhs=xT[:, k * xw + rhs_off : k * xw + rhs_off + 512],
                start=(k == 0),
                stop=(k == KD - 1),
            )
        sl = sb.tile([128, 512], dt.bfloat16, tag="silu")
        nc.scalar.activation(sl[:], ps_g[:], AF.Silu)
        nc.vector.tensor_mul(h_sb[:, i * 512 : (i + 1) * 512], ps_a[:], sl[:])
    for mo in range(KD):
        ps2 = pmm.tile([128, 512], dt.float32, tag="mm")
        for k2 in range(KI):
            nc.tensor.matmul(
                ps2[:],
                lhsT=wr2[k2][:, mo * 128 : (mo + 1) * 128],
                rhs=h_sb[:, k2 * 512 : (k2 + 1) * 512],
                start=(k2 == 0),
                stop=(k2 == KI - 1),
            )
        yo = sb.tile([128, 512], dt.float32, tag="yo")
        if gate_sb is not None:
            nc.vector.tensor_mul(yo[:], ps2[:], gate_sb[:])
        else:
            nc.scalar.activation(yo[:], ps2[:], AF.Copy)
        nc.sync.dma_start(
            out_dram[mo * 128 : (mo + 1) * 128, out_col : out_col + 512], yo[:]
        )


def _emit(nc, tc, t):
    from contextlib import ExitStack

    ctx = ExitStack()
    with ctx:
        const = ctx.enter_context(tc.tile_pool(name="const", bufs=1))
        sb = ctx.enter_context(tc.tile_pool(name="sb", bufs=2))
        sb1 = ctx.enter_context(tc.tile_pool(name="sb1", bufs=1))
        wres = ctx.enter_context(tc.tile_pool(name="wres", bufs=1))
        cw = ctx.enter_context(tc.tile_pool(name="cw", bufs=1))  # [16,*] scratch
        bis = ctx.enter_context(tc.tile_pool(name="bis", bufs=1))
        dr = ctx.enter_context(tc.tile_pool(name="dr", bufs=1, space="DRAM"))
        pmm = ctx.enter_context(tc.tile_pool(name="pmm", bufs=4, space="PSUM"))
        ptr = ctx.enter_context(tc.tile_pool(name="ptr", bufs=2, space="PSUM"))
        psm = ctx.enter_context(tc.tile_pool(name="psm", bufs=2, space="PSUM"))
        pools = {"sb": sb, "sb1": sb1, "wres": wres, "pmm": pmm}

        def load_const(name, shape, dtype=dt.float32):
            tl = const.tile(shape, dtype, tag=name)
            nc.sync.dma_start(tl[:], t[name][:])
            return tl

        esel = load_const("esel", [128, 16])
        bsel = load_const("bsel", [BS, 1])
        b2 = load_const("b2", [16, BS])
        lt16 = load_const("lt16", [16, 16])
        rsmat = load_const("rsmat", [128, 128])
        dsmat = load_const("dsmat", [128, 128])
        ident = load_const("ident", [128, 128])
        ones128 = load_const("ones128", [1, 128])
        iota_tid = load_const("iota_tid", [16, 512], dt.uint16)
        iota_sh = load_const("iota_sh", [128, 64], dt.int16)

        # ================= router =================
        wg_sb = sb1.tile([E, 2 * DIM], dt.float32, tag="h_sb")
        nc.sync.dma_start(wg_sb[:], t["wg"][:])
        wgT = sb1.tile([128, 16 * E], dt.float32, tag="wgT")
        for k in range(16):
            ps = ptr.tile([128, 128], dt.float32, tag="tr")
            nc.tensor.transpose(
                out=ps[:, :E],
                in_=wg_sb[:, k * 128 : (k + 1) * 128],
                identity=ident[:E, :E],
            )
            nc.vector.tensor_copy(wgT[:, k * E : (k + 1) * E], ps[:, :E])

        ts_sb = cw.tile([BS, DIM], dt.float32, tag="cwf")
        nc.sync.dma_start(ts_sb[:], t["ts"][:])
        tsT = sb1.tile([128, KD * BS], dt.float32, tag="tsT")
        for k in range(KD):
            ps = ptr.tile([128, 128], dt.float32, tag="tr")
            nc.tensor.transpose(
                out=ps[:, :BS],
                in_=ts_sb[:, k * 128 : (k + 1) * 128],
                identity=ident[:BS, :BS],
            )
            nc.vector.tensor_copy(tsT[:, k * BS : (k + 1) * BS], ps[:, :BS])

        biasT_ps = psm.tile([BS, E], dt.float32, tag="small")
        for k in range(KD):
            nc.tensor.matmul(
                biasT_ps[:],
                lhsT=tsT[:, k * BS : (k + 1) * BS],
                rhs=wgT[:, k * E : (k + 1) * E],
                start=(k == 0),
                stop=(k == KD - 1),
            )
        biasT_sb = sb1.tile([BS, E], dt.float32, tag="biasT")
        nc.vector.tensor_copy(biasT_sb[:], biasT_ps[:])
        bias_ps = psm.tile([E, 1], dt.float32, tag="small")
        nc.tensor.matmul(bias_ps[:], lhsT=biasT_sb[:], rhs=bsel[:], start=True, stop=True)
        bias_mine = sb1.tile([E, 1], dt.float32, tag="bias_mine")
        nc.vector.tensor_copy(bias_mine[:], bias_ps[:])

        ag_in = dr.tile([E, TSH], dt.float32)
        for n in range(TSH // 512):
            hsuT = sb1.tile([128, KD * 512], dt.float32, tag="h_sb")
            for tt in range(4):
                row = sb.tile([128, DIM], dt.float32, tag="row_in")
                nc.sync.dma_start(
                    row[:], t["hsu_sh"][n * 512 + tt * 128 : n * 512 + (tt + 1) * 128, :]
                )
                for k in range(KD):
                    ps = ptr.tile([128, 128], dt.float32, tag="tr")
                    nc.tensor.transpose(
                        out=ps[:], in_=row[:, k * 128 : (k + 1) * 128], identity=ident[:]
                    )
                    nc.vector.tensor_copy(
                        hsuT[:, k * 512 + tt * 128 : k * 512 + (tt + 1) * 128], ps[:]
                    )
            lps = pmm.tile([128, 512], dt.float32, tag="mm")
            for k in range(KD):
                nc.tensor.matmul(
                    lps[:E, :],
                    lhsT=wgT[:, (8 + k) * E : (9 + k) * E],
                    rhs=hsuT[:, k * 512 : (k + 1) * 512],
                    start=(k == 0),
                    stop=(k == KD - 1),
                )
            lchunk = sb.tile([E, 512], dt.float32, tag="yo")
            nc.vector.tensor_scalar(
                lchunk[:], lps[:E, :], bias_mine[:], None, op0=ALU.add
            )
            nc.sync.dma_start(ag_in[:, n * 512 : (n + 1) * 512], lchunk[:])

        ag_out = dr.tile([NCORES * E, TSH], dt.float32, addr_space="Shared")
        nc.gpsimd.collective_compute(
            "AllGather",
            ALU.bypass,
            replica_groups=[list(range(NCORES))],
            ins=[ag_in[:]],
            outs=[ag_out[:]],
        )
        logit_all = sb1.tile([128, 512], dt.float32, tag="logit_all")
        nc.sync.dma_start(
            logit_all[:], ag_out[:].rearrange("(r e) (c t) -> (r e c) t", e=E, c=2)
        )

        # ============ bf16 copies of hidden_states (for fused gather+transpose)
        hs_sh_bf16 = dr.tile([TSH, DIM], dt.bfloat16)
        for tt in range(TSH // 128):
            row = sb.tile([128, DIM], dt.float32, tag="row_in")
            nc.sync.dma_start(row[:], t["hs_sh"][tt * 128 : (tt + 1) * 128, :])
            rowb = sb.tile([128, DIM], dt.bfloat16, tag="rowb")
            nc.vector.tensor_copy(rowb[:], row[:])
            nc.sync.dma_start(hs_sh_bf16[tt * 128 : (tt + 1) * 128, :], rowb[:])
        hs_bf16 = dr.tile([T, DIM], dt.bfloat16)
        for tt in range(T // 128):
            row = sb.tile([128, DIM], dt.float32, tag="row_in")
            nc.sync.dma_start(row[:], t["hs"][tt * 128 : (tt + 1) * 128, :])
            rowb = sb.tile([128, DIM], dt.bfloat16, tag="rowb")
            nc.vector.tensor_copy(rowb[:], row[:])
            nc.sync.dma_start(hs_bf16[tt * 128 : (tt + 1) * 128, :], rowb[:])

        # ============ shared expert (PE filler during bisection) ============
        wrs1 = _load_weights_bf16(nc, pools, t["ws1"], KD, I2, "wr1_")
        wrs2 = _load_weights_bf16(nc, pools, t["ws2"], KI, DIM, "wr2_")
        for n in range(TSH // 512):
            xsT = sb1.tile([128, KD * 512], dt.bfloat16, tag="xT")
            nc.gpsimd.dma_gather(
                out_ap=xsT[:].rearrange("p (k t) -> p k t", t=512),
                in_ap=hs_sh_bf16[:],
                idxs_ap=iota_sh[:, n * 32 : (n + 1) * 32],
                num_idxs=512,
                num_idxs_reg=512,
                elem_size=DIM,
                transpose=True,
            )
            _ffn_chunk(
                nc, pools, wrs1, wrs2, xsT, 0, t["out_shared"], n * 512, None
            )

        # ================= bisection =================
        lo = sb1.tile([128, 1], dt.float32, tag="lo")
        hi = sb1.tile([128, 1], dt.float32, tag="hi")
        nc.vector.memset(lo[:], -16.0)
        nc.vector.memset(hi[:], 16.0)
        for _ in range(BISECT_ITERS):
            mid = bis.tile([128, 1], dt.float32, tag="mid")
            nc.vector.tensor_add(mid[:], lo[:], hi[:])
            nc.vector.tensor_scalar_mul(mid[:], mid[:], 0.5)
            cmp = bis.tile([128, 512], dt.float32, tag="cmp")
            nc.vector.tensor_scalar(cmp[:], logit_all[:], mid[:], None, op0=ALU.is_ge)
            cnt = bis.tile([128, 1], dt.float32, tag="cnt")
            nc.vector.tensor_reduce(cnt[:], cmp[:], mybir.AxisListType.X, ALU.add)
            cntg_ps = psm.tile([128, 1], dt.float32, tag="small")
            nc.tensor.matmul(
                cntg_ps[:], lhsT=rsmat[:], rhs=cnt[:], start=True, stop=True
            )
            cntg = bis.tile([128, 1], dt.float32, tag="cntg")
            nc.vector.tensor_copy(cntg[:], cntg_ps[:])
            pred = bis.tile([128, 1], dt.uint8, tag="pred")
            nc.vector.tensor_scalar(pred[:], cntg[:], float(CAP), None, op0=ALU.is_ge)
            nc.vector.copy_predicated(lo[:], pred[:], mid[:])
            nc.vector.tensor_scalar(pred[:], cntg[:], float(CAP), None, op0=ALU.is_lt)
            nc.vector.copy_predicated(hi[:], pred[:], mid[:])
        nc.sync.dma_start(t["dbg_thr"][:], lo[:])

        # ================= gates =================
        mask = sb1.tile([128, 512], dt.float32, tag="mask")
        nc.vector.tensor_scalar(mask[:], logit_all[:], lo[:], None, op0=ALU.is_ge)
        sig = sb1.tile([128, 512], dt.float32, tag="sig")
        nc.scalar.activation(sig[:], logit_all[:], AF.Sigmoid)
        g = sb1.tile([128, 512], dt.float32, tag="g")
        nc.vector.tensor_mul(g[:], sig[:], mask[:])
        d_ps = pmm.tile([128, 512], dt.float32, tag="mm")
        nc.tensor.matmul(d_ps[:], lhsT=dsmat[:], rhs=g[:], start=True, stop=True)
        dsafe = sb1.tile([128, 512], dt.float32, tag="dsafe")
        nc.vector.tensor_scalar(dsafe[:], d_ps[:], 1e-12, None, op0=ALU.add)
        drec = sb1.tile([128, 512], dt.float32, tag="drec")
        nc.vector.reciprocal(drec[:], dsafe[:])
        ghat = sb1.tile([128, 512], dt.float32, tag="ghat")
        nc.vector.tensor_mul(ghat[:], g[:], drec[:])

        ghm_ps = psm.tile([16, 512], dt.float32, tag="small")
        nc.tensor.matmul(ghm_ps[:], lhsT=esel[:], rhs=ghat[:], start=True, stop=True)
        ghat_mine = sb1.tile([16, 512], dt.float32, tag="ghat_mine")
        nc.vector.tensor_copy(ghat_mine[:], ghm_ps[:])
        msk_ps = psm.tile([16, 512], dt.float32, tag="small")
        nc.tensor.matmul(msk_ps[:], lhsT=esel[:], rhs=mask[:], start=True, stop=True)
        mask_mine = sb1.tile([16, 512], dt.float32, tag="mask_mine")
        nc.vector.tensor_copy(mask_mine[:], msk_ps[:])

        # ================= compaction =================
        incl = sb1.tile([16, 512], dt.float32, tag="incl")
        nc.vector.tensor_tensor_scan(
            incl[:], mask_mine[:], mask_mine[:], 0.0, op0=ALU.add, op1=ALU.bypass
        )
        offs_ps = psm.tile([16, 1], dt.float32, tag="small")
        nc.tensor.matmul(
            offs_ps[:], lhsT=lt16[:], rhs=incl[:, 511:512], start=True, stop=True
        )
        # pos (within-b slot) = (incl - mask) + offs - b*CAP
        pos = sb1.tile([16, 512], dt.float32, tag="pos")
        nc.vector.tensor_sub(pos[:], incl[:], mask_mine[:])
        offs = cw.tile([16, 1], dt.float32, tag="cwo")
        nc.vector.tensor_copy(offs[:], offs_ps[:])
        nc.vector.tensor_scalar(pos[:], pos[:], offs[:], None, op0=ALU.add)
        boff = cw.tile([16, 1], dt.float32, tag="cwo")
        nc.vector.tensor_scalar(boff[:], b2[:, 1:2], float(CAP), None, op0=ALU.mult)
        nc.vector.tensor_scalar(pos[:], pos[:], boff[:], None, op0=ALU.subtract)
        # invalid/overflow -> -1:  p1 = (pos+1) * ok - 1
        okm = cw.tile([16, 512], dt.float32, tag="cwa")
        nc.vector.tensor_scalar(okm[:], pos[:], float(CAP - 1), None, op0=ALU.is_le)
        nc.vector.tensor_mul(okm[:], okm[:], mask_mine[:])
        p1 = cw.tile([16, 512], dt.float32, tag="cwb")
        nc.vector.tensor_scalar(p1[:], pos[:], 1.0, None, op0=ALU.add)
        nc.vector.tensor_mul(p1[:], p1[:], okm[:])
        nc.vector.tensor_scalar(p1[:], p1[:], 1.0, None, op0=ALU.subtract)
        pos_i16 = sb1.tile([16, 512], dt.int16, tag="pos_i16")
        nc.vector.tensor_copy(pos_i16[:], p1[:])

        gbits = ghat_mine[:].bitcast(dt.uint16).rearrange("p (t two) -> p t two", two=2)
        glo = sb1.tile([16, 512], dt.uint16, tag="glo")
        nc.vector.tensor_copy(glo[:, :, None], gbits[:, :, 0:1])
        ghi = sb1.tile([16, 512], dt.uint16, tag="ghi")
        nc.vector.tensor_copy(ghi[:, :, None], gbits[:, :, 1:2])

        cc = {}
        for name, data in (("tid", iota_tid), ("ghi", ghi), ("glo", glo)):
            so = cw.tile([16, CAP], dt.uint16, tag="cws")
            nc.gpsimd.local_scatter(
                out_ap=so[:],
                data_ap=data[:],
                idxs_ap=pos_i16[:],
                channels=16,
                num_elems=CAP,
                num_idxs=512,
            )
            sf = cw.tile([16, CAP], dt.float32, tag="cwf")
            nc.vector.tensor_copy(sf[:], so[:])
            # collapse 16 partitions -> [2, CAP] (one row per batch)
            ccn = sb1.tile([BS, CAP], dt.float32, tag=f"cc_{name}")
            for h in range(2):
                cps = psm.tile([BS, 512], dt.float32, tag="small")
                nc.tensor.matmul(
                    cps[:],
                    lhsT=b2[:],
                    rhs=sf[:, h * 512 : (h + 1) * 512],
                    start=True,
                    stop=True,
                )
                nc.vector.tensor_copy(ccn[:, h * 512 : (h + 1) * 512], cps[:])
            cc[name] = ccn

        # gates: f32 bits = ghi*65536 + glo (int32, exact)
        c64k = cw.tile([BS, 1], dt.int32, tag="c64k")
        nc.vector.memset(c64k[:], 65536)
        glo_i = cw.tile([BS, CAP], dt.int32, tag="cwh")
        nc.vector.tensor_copy(glo_i[:], cc["glo"][:])
        gbits_i = sb1.tile([BS, CAP], dt.int32, tag="gbits_i")
        nc.vector.tensor_copy(gbits_i[:], cc["ghi"][:])
        nc.vector.tensor_tensor(
            gbits_i[:], gbits_i[:], c64k[:].to_broadcast([BS, CAP]), op=ALU.mult
        )
        nc.vector.tensor_add(gbits_i[:], gbits_i[:], glo_i[:])
        gatec = gbits_i[:].bitcast(dt.float32)
        nc.sync.dma_start(t["dbg_gate"][:], gatec)
        gate_buf = dr.tile([BS, CAP], dt.float32)
        nc.sync.dma_start(gate_buf[:], gatec)

        tid_i = cw.tile([BS, CAP], dt.int32, tag="cwh")
        nc.vector.tensor_copy(tid_i[:], cc["tid"][:])
        nc.sync.dma_start(
            t["out_idx"][:].rearrange("(b t) one -> b (t one)", b=BS), tid_i[:]
        )
        idx_buf = dr.tile([SLOTS, 1], dt.int32)
        nc.sync.dma_start(
            idx_buf[:].rearrange("(b t) one -> b (t one)", b=BS), tid_i[:]
        )

        # ================= dispatch + expert FFN =================
        wr1 = _load_weights_bf16(nc, pools, t["w1"], KD, I2, "wr1_")
        wr2 = _load_weights_bf16(nc, pools, t["w2"], KI, DIM, "wr2_")
        # int16 slot->token table to DRAM for the gather engine
        tid_i16 = cw.tile([BS, CAP], dt.int16, tag="cws")
        nc.vector.tensor_copy(tid_i16[:], cc["tid"][:])
        idx16_buf = dr.tile([SLOTS, 1], dt.int16)
        nc.sync.dma_start(
            idx16_buf[:].rearrange("(b t) one -> b (t one)", b=BS), tid_i16[:]
        )
        # wrapped view: chunk n, slot i -> [i%16, n*32 + i//16]
        idx16_w = idx16_buf[:].rearrange("(n c p) one -> p (n c one)", p=16, c=32)
        for n in range(SLOTS // 512):
            idxw = sb.tile([128, 32], dt.int16, tag="idxw")
            for rep in range(8):
                nc.sync.dma_start(
                    idxw[rep * 16 : (rep + 1) * 16, :],
                    idx16_w[:, n * 32 : (n + 1) * 32],
                )
            xT = sb1.tile([128, KD * 512], dt.bfloat16, tag="xT")
            nc.gpsimd.dma_gather(
                out_ap=xT[:].rearrange("p (k t) -> p k t", t=512),
                in_ap=hs_bf16[:],
                idxs_ap=idxw[:],
                num_idxs=512,
                num_idxs_reg=512,
                elem_size=DIM,
                transpose=True,
            )
            grow = sb1.tile([1, 512], dt.float32, tag="grow")
            nc.sync.dma_start(
                grow[:],
                gate_buf[:].rearrange("b (m t) -> (b m) t", t=512)[n : n + 1, :],
            )
            grep_ps = pmm.tile([128, 512], dt.float32, tag="mm")
            nc.tensor.matmul(
                grep_ps[:], lhsT=ones128[:], rhs=grow[:], start=True, stop=True
            )
            gate_sb = sb1.tile([128, 512], dt.float32, tag="gate_sb")
            nc.vector.tensor_copy(gate_sb[:], grep_ps[:])
            _ffn_chunk(
                nc, pools, wr1, wr2, xT, 0, t["out_routed"], n * 512, gate_sb
            )


# ======================= host side =======================

_CACHED_NC = None


def _get_nc():
    global _CACHED_NC
    if _CACHED_NC is None:
        _CACHED_NC = build_nc()
    return _CACHED_NC


def make_in_maps(inputs):
    hs_flat = np.ascontiguousarray(
        np.asarray(inputs["hidden_states"], dtype=np.float32).reshape(T, DIM)
    )
    hsu_flat = np.ascontiguousarray(
        np.asarray(inputs["hidden_states_unmodulated"], dtype=np.float32).reshape(
            T, DIM
        )
    )
    ts = np.asarray(inputs["timestep"], dtype=np.float32)
    Wg = np.asarray(inputs["Wg"], dtype=np.float32)
    W1 = np.asarray(inputs["W1"], dtype=np.float32)
    W2 = np.asarray(inputs["W2"], dtype=np.float32)
    Ws1 = np.ascontiguousarray(np.asarray(inputs["Ws1"], dtype=np.float32))
    Ws2 = np.ascontiguousarray(np.asarray(inputs["Ws2"], dtype=np.float32))

    lt16 = np.triu(np.ones((16, 16), np.float32), 1)  # lhsT[k,m]=1 iff k<m
    b2 = np.zeros((16, BS), np.float32)
    b2[:8, 0] = 1.0
    b2[8:, 1] = 1.0
    # partition layout: p = r*16 + e*2 + c  (r = source core, e = expert,
    # c = 512-token half of the core's shard)
    p = np.arange(128)
    pb = p // 64  # batch  (r//4)
    pe = (p % 16) // 2  # expert
    ptok = p // 16 * 2 + p % 2  # token-chunk id (r*2 + c)
    rsmat = ((pb[:, None] == pb[None, :]) & (pe[:, None] == pe[None, :])).astype(
        np.float32
    )
    dsmat = (ptok[:, None] == ptok[None, :]).astype(np.float32)
    ident = np.eye(128, dtype=np.float32)
    ones128 = np.ones((1, 128), np.float32)
    j = np.arange(16)[:, None]
    tt = np.arange(512)[None, :]
    iota_tid = (j * 512 + tt).astype(np.uint16)
    # wrapped dispatch indices for the shared shard: i -> [i%16, i//16], x8 replicas
    ii = np.arange(TSH)
    iw = np.zeros((16, TSH // 16), np.int16)
    iw[ii % 16, ii // 16] = ii

    in_maps = []
    for c in range(NCORES):
        # extract my expert's 16 rows in (b-major, chunk) order:
        # j = r*2 + cc  ->  partition (j//2)*16 + c*2 + (j%2)
        esel = np.zeros((128, 16), np.float32)
        for j in range(16):
            esel[(j // 2) * 16 + c * 2 + (j % 2), j] = 1.0
        bsel = np.zeros((BS, 1), np.float32)
        bsel[c // 4, 0] = 1.0
        in_maps.append(
            {
                "hs": hs_flat,
                "hs_sh": np.ascontiguousarray(hs_flat[c * TSH : (c + 1) * TSH]),
                "hsu_sh": np.ascontiguousarray(hsu_flat[c * TSH : (c + 1) * TSH]),
                "ts": ts,
                "wg": Wg,
                "w1": np.ascontiguousarray(W1[c]),
                "w2": np.ascontiguousarray(W2[c]),
                "ws1": Ws1,
                "ws2": Ws2,
                "esel": esel,
                "bsel": bsel,
                "b2": b2,
                "lt16": lt16,
                "rsmat": rsmat,
                "dsmat": dsmat,
                "ident": ident,
                "ones128": ones128,
                "iota_tid": iota_tid,
                "iota_sh": np.tile(iw, (8, 1)),
            }
        )
    return in_maps


def combine(results):
    out = np.empty((T, DIM), np.float32)
    for c in range(NCORES):
        out[c * TSH : (c + 1) * TSH] = results[c]["out_shared"].T
    for c in range(NCORES):
        idx = results[c]["out_idx"].reshape(SLOTS)
        out[idx] += results[c]["out_routed"].T
    return out.reshape(BS, SLEN, DIM)


def kernel(**inputs):
    nc = _get_nc()
    in_maps = make_in_maps(inputs)
    res = run_bass_kernel_spmd(nc, in_maps, list(range(NCORES))).results
    return combine(res)


if __name__ == "__main__":
    nc = build_nc()
    print("build ok:", len(nc.inst_map), "instructions")


# revision 20
# speedup vs baseline: 1.2776x; 1.2776x over previous
"""Expert-choice MoE layer (NucleusMoELayer) on 8 Trainium2 NeuronCores.

Strategy (expert-parallel):
 - one expert per core; router + gate-normalization replicated from an
   AllGathered logit table; shared expert sharded over tokens (1024/core)
 - router logits for the core's own 1024-token shard via fp32 PE matmul,
   AllGather -> every core sees all [8 experts x 8192 tokens] logits
 - expert-choice top-1024-per-(batch,expert) via 32-step threshold bisection
   on logits (sigmoid is monotone so logit ranking == score ranking)
 - compaction (selected tokens -> dense slots) via per-partition cumsum +
   GpSimd local_scatter of (token-id, gate-hi, gate-lo) uint16 payloads
 - dispatch: indirect-DMA row gather of selected tokens, PE-transpose to
   feature-major, swiglu FFN in bf16 (weights streamed + cast on the fly)
 - outputs are feature-major; host transposes, concatenates shared shards and
   scatter-adds routed outputs (unique indices per core)

kernel(**inputs) takes FULL unsharded inputs, returns the FULL output.
"""

import sys

if "/opt/trn_rl_repo" not in sys.path:
    sys.path.insert(0, "/opt/trn_rl_repo")

import numpy as np

import concourse.bacc as bacc
import concourse.bass as bass
import concourse.mybir as mybir
import concourse.tile as tile
from concourse.bass_utils import run_bass_kernel_spmd

dt = mybir.dt
AF = mybir.ActivationFunctionType
ALU = mybir.AluOpType

NCORES = 8
BS, SLEN, DIM = 2, 4096, 1024
INNER = 2048
I2 = 2 * INNER  # 4096
E = 8
CAP = 1024  # tokens per (batch, expert)
T = BS * SLEN  # 8192 global tokens
TSH = T // NCORES  # 1024-token shard per core
SLOTS = BS * CAP  # 2048 routed slots per expert
KD = DIM // 128  # 8 k-chunks over dim
MI = I2 // 128  # 32 m-chunks over 2*inner
KI = INNER // 128  # 16 k-chunks over inner
BISECT_ITERS = 32


def build_nc():
    nc = bacc.Bacc(None, target_bir_lowering=False, num_devices=NCORES)

    tens = {}

    def din(name, shape, dtype=dt.float32):
        tens[name] = nc.dram_tensor(name, shape, dtype, kind="ExternalInput")

    def dout(name, shape, dtype=dt.float32):
        tens[name] = nc.dram_tensor(name, shape, dtype, kind="ExternalOutput")

    din("hs_b", [T, DIM], dt.bfloat16)
    din("hs_sh_b", [TSH, DIM], dt.bfloat16)
    din("hsu_sh", [TSH, DIM])
    din("ts", [BS, DIM])
    din("wg", [E, 2 * DIM])
    din("w1_b", [DIM, I2], dt.bfloat16)
    din("w2_b", [INNER, DIM], dt.bfloat16)
    din("ws1_b", [DIM, I2], dt.bfloat16)
    din("ws2_b", [INNER, DIM], dt.bfloat16)
    din("esel", [128, 16])
    din("bsel", [BS, 1])
    din("b2", [16, BS])
    din("lt16", [16, 16])
    din("rsmat", [128, 128])
    din("dsmat", [128, 128])
    din("ident", [128, 128])
    din("ones128", [1, 128])
    din("iota_tid", [16, 512], dt.uint16)
    din("iota_sh", [128, 64], dt.int16)
    dout("out_routed", [DIM, SLOTS])
    dout("out_idx", [SLOTS, 1], dt.int32)
    dout("out_shared", [DIM, TSH])
    dout("dbg_thr", [128, 1])
    dout("dbg_gate", [BS, CAP])

    with tile.TileContext(nc, num_cores=NCORES) as tc:
        _emit(nc, tc, tens)
    nc.finalize()
    return nc


def _load_weights_bf16(nc, pools, wdram, nk, width, tagp):
    """Load [nk*128, width] bf16 DRAM weights into resident k-tiles."""
    wr_pool = pools["wres"]
    tiles = []
    for k in range(nk):
        wt = wr_pool.tile([128, width], dt.bfloat16, tag=f"{tagp}{k}")
        nc.sync.dma_start(wt[:], wdram[k * 128 : (k + 1) * 128, :])
        tiles.append(wt)
    return tiles


def _ffn_chunk(nc, pools, wr1, wr2, xT, rhs_off, out_dram, out_col, gate_sb):
    """One 512-token swiglu FFN chunk, feature-major, resident bf16 weights.
    wr1: 8 k-tiles [128, I2]; wr2: 16 k-tiles [128, DIM]; xT [128, KD*xw]."""
    sb, sb1, pmm = pools["sb"], pools["sb1"], pools["pmm"]
    xw = xT.shape[1] // KD
    h_sb = sb1.tile([128, 16 * 512], dt.bfloat16, tag="h_sb")
    for i in range(16):
        ps_a = pmm.tile([128, 512], dt.float32, tag="mm")
        for k in range(KD):
            nc.tensor.matmul(
                ps_a[:],
                lhsT=wr1[k][:, i * 128 : (i + 1) * 128],
                rhs=xT[:, k * xw + rhs_off : k * xw + rhs_off + 512],
                start=(k == 0),
                stop=(k == KD - 1),
            )
        ps_g = pmm.tile([128, 512], dt.float32, tag="mm")
        for k in range(KD):
            nc.tensor.matmul(
                ps_g[:],
                lhsT=wr1[k][:, (16 + i) * 128 : (17 + i) * 128],
                rhs=xT[:, k * xw + rhs_off : k * xw + rhs_off + 512],
                start=(k == 0),
                stop=(k == KD - 1),
            )
        sl = sb.tile([128, 512], dt.bfloat16, tag="silu")
        nc.scalar.activation(sl[:], ps_g[:], AF.Silu)
        nc.vector.tensor_mul(h_sb[:, i * 512 : (i + 1) * 512], ps_a[:], sl[:])
    for mo in range(KD):
        ps2 = pmm.tile([128, 512], dt.float32, tag="mm")
        for k2 in range(KI):
            nc.tensor.matmul(
                ps2[:],
                lhsT=wr2[k2][:, mo * 128 : (mo + 1) * 128],
                rhs=h_sb[:, k2 * 512 : (k2 + 1) * 512],
                start=(k2 == 0),
                stop=(k2 == KI - 1),
            )
        yo = sb.tile([128, 512], dt.float32, tag="yo")
        if gate_sb is not None:
            nc.vector.tensor_mul(yo[:], ps2[:], gate_sb[:])
        else:
            nc.scalar.activation(yo[:], ps2[:], AF.Copy)
        nc.sync.dma_start(
            out_dram[mo * 128 : (mo + 1) * 128, out_col : out_col + 512], yo[:]
        )


def _emit(nc, tc, t):
    from contextlib import ExitStack

    ctx = ExitStack()
    with ctx:
        const = ctx.enter_context(tc.tile_pool(name="const", bufs=1))
        sb = ctx.enter_context(tc.tile_pool(name="sb", bufs=2))
        sb1 = ctx.enter_context(tc.tile_pool(name="sb1", bufs=1))
        wres = ctx.enter_context(tc.tile_pool(name="wres", bufs=1))
        cw = ctx.enter_context(tc.tile_pool(name="cw", bufs=1))  # [16,*] scratch
        bis = ctx.enter_context(tc.tile_pool(name="bis", bufs=1))
        dr = ctx.enter_context(tc.tile_pool(name="dr", bufs=1, space="DRAM"))
        pmm = ctx.enter_context(tc.tile_pool(name="pmm", bufs=4, space="PSUM"))
        ptr = ctx.enter_context(tc.tile_pool(name="ptr", bufs=2, space="PSUM"))
        psm = ctx.enter_context(tc.tile_pool(name="psm", bufs=2, space="PSUM"))
        pools = {"sb": sb, "sb1": sb1, "wres": wres, "pmm": pmm}

        def load_const(name, shape, dtype=dt.float32):
            tl = const.tile(shape, dtype, tag=name)
            nc.sync.dma_start(tl[:], t[name][:])
            return tl

        esel = load_const("esel", [128, 16])
        bsel = load_const("bsel", [BS, 1])
        b2 = load_const("b2", [16, BS])
        lt16 = load_const("lt16", [16, 16])
        rsmat = load_const("rsmat", [128, 128])
        dsmat = load_const("dsmat", [128, 128])
        ident = load_const("ident", [128, 128])
        ones128 = load_const("ones128", [1, 128])
        iota_tid = load_const("iota_tid", [16, 512], dt.uint16)
        iota_sh = load_const("iota_sh", [128, 64], dt.int16)

        # ================= router =================
        wg_sb = sb1.tile([E, 2 * DIM], dt.float32, tag="h_sb")
        nc.sync.dma_start(wg_sb[:], t["wg"][:])
        wgT = sb1.tile([128, 16 * E], dt.float32, tag="wgT")
        for k in range(16):
            ps = ptr.tile([128, 128], dt.float32, tag="tr")
            nc.tensor.transpose(
                out=ps[:, :E],
                in_=wg_sb[:, k * 128 : (k + 1) * 128],
                identity=ident[:E, :E],
            )
            nc.vector.tensor_copy(wgT[:, k * E : (k + 1) * E], ps[:, :E])

        ts_sb = cw.tile([BS, DIM], dt.float32, tag="cwf")
        nc.sync.dma_start(ts_sb[:], t["ts"][:])
        tsT = sb1.tile([128, KD * BS], dt.float32, tag="tsT")
        for k in range(KD):
            ps = ptr.tile([128, 128], dt.float32, tag="tr")
            nc.tensor.transpose(
                out=ps[:, :BS],
                in_=ts_sb[:, k * 128 : (k + 1) * 128],
                identity=ident[:BS, :BS],
            )
            nc.vector.tensor_copy(tsT[:, k * BS : (k + 1) * BS], ps[:, :BS])

        biasT_ps = psm.tile([BS, E], dt.float32, tag="small")
        for k in range(KD):
            nc.tensor.matmul(
                biasT_ps[:],
                lhsT=tsT[:, k * BS : (k + 1) * BS],
                rhs=wgT[:, k * E : (k + 1) * E],
                start=(k == 0),
                stop=(k == KD - 1),
            )
        biasT_sb = sb1.tile([BS, E], dt.float32, tag="biasT")
        nc.vector.tensor_copy(biasT_sb[:], biasT_ps[:])
        bias_ps = psm.tile([E, 1], dt.float32, tag="small")
        nc.tensor.matmul(bias_ps[:], lhsT=biasT_sb[:], rhs=bsel[:], start=True, stop=True)
        bias_mine = sb1.tile([E, 1], dt.float32, tag="bias_mine")
        nc.vector.tensor_copy(bias_mine[:], bias_ps[:])

        ag_in = dr.tile([E, TSH], dt.float32)
        for n in range(TSH // 512):
            hsuT = sb1.tile([128, KD * 512], dt.float32, tag="h_sb")
            for tt in range(4):
                row = sb.tile([128, DIM], dt.float32, tag="row_in")
                nc.sync.dma_start(
                    row[:], t["hsu_sh"][n * 512 + tt * 128 : n * 512 + (tt + 1) * 128, :]
                )
                for k in range(KD):
                    ps = ptr.tile([128, 128], dt.float32, tag="tr")
                    nc.tensor.transpose(
                        out=ps[:], in_=row[:, k * 128 : (k + 1) * 128], identity=ident[:]
                    )
                    nc.vector.tensor_copy(
                        hsuT[:, k * 512 + tt * 128 : k * 512 + (tt + 1) * 128], ps[:]
                    )
            lps = pmm.tile([128, 512], dt.float32, tag="mm")
            for k in range(KD):
                nc.tensor.matmul(
                    lps[:E, :],
                    lhsT=wgT[:, (8 + k) * E : (9 + k) * E],
                    rhs=hsuT[:, k * 512 : (k + 1) * 512],
                    start=(k == 0),
                    stop=(k == KD - 1),
                )
            lchunk = sb.tile([E, 512], dt.float32, tag="yo")
            nc.vector.tensor_scalar(
                lchunk[:], lps[:E, :], bias_mine[:], None, op0=ALU.add
            )
            nc.sync.dma_start(ag_in[:, n * 512 : (n + 1) * 512], lchunk[:])

        ag_out = dr.tile([NCORES * E, TSH], dt.float32, addr_space="Shared")
        nc.gpsimd.collective_compute(
            "AllGather",
            ALU.bypass,
            replica_groups=[list(range(NCORES))],
            ins=[ag_in[:]],
            outs=[ag_out[:]],
        )
        logit_all = sb1.tile([128, 512], dt.float32, tag="logit_all")
        nc.sync.dma_start(
            logit_all[:], ag_out[:].rearrange("(r e) (c t) -> (r e c) t", e=E, c=2)
        )

        # ============ shared expert (PE filler during bisection) ============
        wrs1 = _load_weights_bf16(nc, pools, t["ws1_b"], KD, I2, "wr1_")
        wrs2 = _load_weights_bf16(nc, pools, t["ws2_b"], KI, DIM, "wr2_")
        for n in range(TSH // 512):
            xsT = sb1.tile([128, KD * 512], dt.bfloat16, tag="xT")
            nc.gpsimd.dma_gather(
                out_ap=xsT[:].rearrange("p (k t) -> p k t", t=512),
                in_ap=t["hs_sh_b"][:],
                idxs_ap=iota_sh[:, n * 32 : (n + 1) * 32],
                num_idxs=512,
                num_idxs_reg=512,
                elem_size=DIM,
                transpose=True,
            )
            _ffn_chunk(
                nc, pools, wrs1, wrs2, xsT, 0, t["out_shared"], n * 512, None
            )

        # ================= bisection =================
        lo = sb1.tile([128, 1], dt.float32, tag="lo")
        hi = sb1.tile([128, 1], dt.float32, tag="hi")
        nc.vector.memset(lo[:], -16.0)
        nc.vector.memset(hi[:], 16.0)
        for _ in range(BISECT_ITERS):
            mid = bis.tile([128, 1], dt.float32, tag="mid")
            nc.vector.tensor_add(mid[:], lo[:], hi[:])
            nc.vector.tensor_scalar_mul(mid[:], mid[:], 0.5)
            cmp = bis.tile([128, 512], dt.float32, tag="cmp")
            nc.vector.tensor_scalar(cmp[:], logit_all[:], mid[:], None, op0=ALU.is_ge)
            cnt = bis.tile([128, 1], dt.float32, tag="cnt")
            nc.vector.tensor_reduce(cnt[:], cmp[:], mybir.AxisListType.X, ALU.add)
            cntg_ps = psm.tile([128, 1], dt.float32, tag="small")
            nc.tensor.matmul(
                cntg_ps[:], lhsT=rsmat[:], rhs=cnt[:], start=True, stop=True
            )
            cntg = bis.tile([128, 1], dt.float32, tag="cntg")
            nc.vector.tensor_copy(cntg[:], cntg_ps[:])
            pred = bis.tile([128, 1], dt.uint8, tag="pred")
            nc.vector.tensor_scalar(pred[:], cntg[:], float(CAP), None, op0=ALU.is_ge)
            nc.vector.copy_predicated(lo[:], pred[:], mid[:])
            nc.vector.tensor_scalar(pred[:], cntg[:], float(CAP), None, op0=ALU.is_lt)
            nc.vector.copy_predicated(hi[:], pred[:], mid[:])
        nc.sync.dma_start(t["dbg_thr"][:], lo[:])

        # ================= gates =================
        mask = sb1.tile([128, 512], dt.float32, tag="mask")
        nc.vector.tensor_scalar(mask[:], logit_all[:], lo[:], None, op0=ALU.is_ge)
        sig = sb1.tile([128, 512], dt.float32, tag="sig")
        nc.scalar.activation(sig[:], logit_all[:], AF.Sigmoid)
        g = sb1.tile([128, 512], dt.float32, tag="g")
        nc.vector.tensor_mul(g[:], sig[:], mask[:])
        d_ps = pmm.tile([128, 512], dt.float32, tag="mm")
        nc.tensor.matmul(d_ps[:], lhsT=dsmat[:], rhs=g[:], start=True, stop=True)
        dsafe = sb1.tile([128, 512], dt.float32, tag="dsafe")
        nc.vector.tensor_scalar(dsafe[:], d_ps[:], 1e-12, None, op0=ALU.add)
        drec = sb1.tile([128, 512], dt.float32, tag="drec")
        nc.vector.reciprocal(drec[:], dsafe[:])
        ghat = sb1.tile([128, 512], dt.float32, tag="ghat")
        nc.vector.tensor_mul(ghat[:], g[:], drec[:])

        ghm_ps = psm.tile([16, 512], dt.float32, tag="small")
        nc.tensor.matmul(ghm_ps[:], lhsT=esel[:], rhs=ghat[:], start=True, stop=True)
        ghat_mine = sb1.tile([16, 512], dt.float32, tag="ghat_mine")
        nc.vector.tensor_copy(ghat_mine[:], ghm_ps[:])
        msk_ps = psm.tile([16, 512], dt.float32, tag="small")
        nc.tensor.matmul(msk_ps[:], lhsT=esel[:], rhs=mask[:], start=True, stop=True)
        mask_mine = sb1.tile([16, 512], dt.float32, tag="mask_mine")
        nc.vector.tensor_copy(mask_mine[:], msk_ps[:])

        # ================= compaction =================
        incl = sb1.tile([16, 512], dt.float32, tag="incl")
        nc.vector.tensor_tensor_scan(
            incl[:], mask_mine[:], mask_mine[:], 0.0, op0=ALU.add, op1=ALU.bypass
        )
        offs_ps = psm.tile([16, 1], dt.float32, tag="small")
        nc.tensor.matmul(
            offs_ps[:], lhsT=lt16[:], rhs=incl[:, 511:512], start=True, stop=True
        )
        # pos (within-b slot) = (incl - mask) + offs - b*CAP
        pos = sb1.tile([16, 512], dt.float32, tag="pos")
        nc.vector.tensor_sub(pos[:], incl[:], mask_mine[:])
        offs = cw.tile([16, 1], dt.float32, tag="cwo")
        nc.vector.tensor_copy(offs[:], offs_ps[:])
        nc.vector.tensor_scalar(pos[:], pos[:], offs[:], None, op0=ALU.add)
        boff = cw.tile([16, 1], dt.float32, tag="cwo")
        nc.vector.tensor_scalar(boff[:], b2[:, 1:2], float(CAP), None, op0=ALU.mult)
        nc.vector.tensor_scalar(pos[:], pos[:], boff[:], None, op0=ALU.subtract)
        # invalid/overflow -> -1:  p1 = (pos+1) * ok - 1
        okm = cw.tile([16, 512], dt.float32, tag="cwa")
        nc.vector.tensor_scalar(okm[:], pos[:], float(CAP - 1), None, op0=ALU.is_le)
        nc.vector.tensor_mul(okm[:], okm[:], mask_mine[:])
        p1 = cw.tile([16, 512], dt.float32, tag="cwb")
        nc.vector.tensor_scalar(p1[:], pos[:], 1.0, None, op0=ALU.add)
        nc.vector.tensor_mul(p1[:], p1[:], okm[:])
        nc.vector.tensor_scalar(p1[:], p1[:], 1.0, None, op0=ALU.subtract)
        pos_i16 = sb1.tile([16, 512], dt.int16, tag="pos_i16")
        nc.vector.tensor_copy(pos_i16[:], p1[:])

        gbits = ghat_mine[:].bitcast(dt.uint16).rearrange("p (t two) -> p t two", two=2)
        glo = sb1.tile([16, 512], dt.uint16, tag="glo")
        nc.vector.tensor_copy(glo[:, :, None], gbits[:, :, 0:1])
        ghi = sb1.tile([16, 512], dt.uint16, tag="ghi")
        nc.vector.tensor_copy(ghi[:, :, None], gbits[:, :, 1:2])

        cc = {}
        for name, data in (("tid", iota_tid), ("ghi", ghi), ("glo", glo)):
            so = cw.tile([16, CAP], dt.uint16, tag="cws")
            nc.gpsimd.local_scatter(
                out_ap=so[:],
                data_ap=data[:],
                idxs_ap=pos_i16[:],
                channels=16,
                num_elems=CAP,
                num_idxs=512,
            )
            sf = cw.tile([16, CAP], dt.float32, tag="cwf")
            nc.vector.tensor_copy(sf[:], so[:])
            # collapse 16 partitions -> [2, CAP] (one row per batch)
            ccn = sb1.tile([BS, CAP], dt.float32, tag=f"cc_{name}")
            for h in range(2):
                cps = psm.tile([BS, 512], dt.float32, tag="small")
                nc.tensor.matmul(
                    cps[:],
                    lhsT=b2[:],
                    rhs=sf[:, h * 512 : (h + 1) * 512],
                    start=True,
                    stop=True,
                )
                nc.vector.tensor_copy(ccn[:, h * 512 : (h + 1) * 512], cps[:])
            cc[name] = ccn

        # gates: f32 bits = ghi*65536 + glo (int32, exact)
        c64k = cw.tile([BS, 1], dt.int32, tag="c64k")
        nc.vector.memset(c64k[:], 65536)
        glo_i = cw.tile([BS, CAP], dt.int32, tag="cwh")
        nc.vector.tensor_copy(glo_i[:], cc["glo"][:])
        gbits_i = sb1.tile([BS, CAP], dt.int32, tag="gbits_i")
        nc.vector.tensor_copy(gbits_i[:], cc["ghi"][:])
        nc.vector.tensor_tensor(
            gbits_i[:], gbits_i[:], c64k[:].to_broadcast([BS, CAP]), op=ALU.mult
        )
        nc.vector.tensor_add(gbits_i[:], gbits_i[:], glo_i[:])
        gatec = gbits_i[:].bitcast(dt.float32)
        nc.sync.dma_start(t["dbg_gate"][:], gatec)
        gate_buf = dr.tile([BS, CAP], dt.float32)
        nc.sync.dma_start(gate_buf[:], gatec)

        tid_i = cw.tile([BS, CAP], dt.int32, tag="cwh")
        nc.vector.tensor_copy(tid_i[:], cc["tid"][:])
        nc.sync.dma_start(
            t["out_idx"][:].rearrange("(b t) one -> b (t one)", b=BS), tid_i[:]
        )
        idx_buf = dr.tile([SLOTS, 1], dt.int32)
        nc.sync.dma_start(
            idx_buf[:].rearrange("(b t) one -> b (t one)", b=BS), tid_i[:]
        )

        # ================= dispatch + expert FFN =================
        wr1 = _load_weights_bf16(nc, pools, t["w1_b"], KD, I2, "wr1_")
        wr2 = _load_weights_bf16(nc, pools, t["w2_b"], KI, DIM, "wr2_")
        # int16 slot->token table to DRAM for the gather engine
        tid_i16 = cw.tile([BS, CAP], dt.int16, tag="cws")
        nc.vector.tensor_copy(tid_i16[:], cc["tid"][:])
        idx16_buf = dr.tile([SLOTS, 1], dt.int16)
        nc.sync.dma_start(
            idx16_buf[:].rearrange("(b t) one -> b (t one)", b=BS), tid_i16[:]
        )
        # wrapped view: chunk n, slot i -> [i%16, n*32 + i//16]
        idx16_w = idx16_buf[:].rearrange("(n c p) one -> p (n c one)", p=16, c=32)
        for n in range(SLOTS // 512):
            idxw = sb.tile([128, 32], dt.int16, tag="idxw")
            for rep in range(8):
                nc.sync.dma_start(
                    idxw[rep * 16 : (rep + 1) * 16, :],
                    idx16_w[:, n * 32 : (n + 1) * 32],
                )
            xT = sb1.tile([128, KD * 512], dt.bfloat16, tag="xT")
            nc.gpsimd.dma_gather(
                out_ap=xT[:].rearrange("p (k t) -> p k t", t=512),
                in_ap=t["hs_b"][:],
                idxs_ap=idxw[:],
                num_idxs=512,
                num_idxs_reg=512,
                elem_size=DIM,
                transpose=True,
            )
            grow = sb1.tile([1, 512], dt.float32, tag="grow")
            nc.sync.dma_start(
                grow[:],
                gate_buf[:].rearrange("b (m t) -> (b m) t", t=512)[n : n + 1, :],
            )
            grep_ps = pmm.tile([128, 512], dt.float32, tag="mm")
            nc.tensor.matmul(
                grep_ps[:], lhsT=ones128[:], rhs=grow[:], start=True, stop=True
            )
            gate_sb = sb1.tile([128, 512], dt.float32, tag="gate_sb")
            nc.vector.tensor_copy(gate_sb[:], grep_ps[:])
            _ffn_chunk(
                nc, pools, wr1, wr2, xT, 0, t["out_routed"], n * 512, gate_sb
            )


# ======================= host side =======================

_CACHED_NC = None


def _get_nc():
    global _CACHED_NC
    if _CACHED_NC is None:
        _CACHED_NC = build_nc()
    return _CACHED_NC


def make_in_maps(inputs):
    hs_flat = np.ascontiguousarray(
        np.asarray(inputs["hidden_states"], dtype=np.float32).reshape(T, DIM)
    )
    hsu_flat = np.ascontiguousarray(
        np.asarray(inputs["hidden_states_unmodulated"], dtype=np.float32).reshape(
            T, DIM
        )
    )
    ts = np.asarray(inputs["timestep"], dtype=np.float32)
    Wg = np.asarray(inputs["Wg"], dtype=np.float32)
    W1 = np.asarray(inputs["W1"], dtype=np.float32)
    W2 = np.asarray(inputs["W2"], dtype=np.float32)
    Ws1 = np.ascontiguousarray(np.asarray(inputs["Ws1"], dtype=np.float32))
    Ws2 = np.ascontiguousarray(np.asarray(inputs["Ws2"], dtype=np.float32))

    lt16 = np.triu(np.ones((16, 16), np.float32), 1)  # lhsT[k,m]=1 iff k<m
    b2 = np.zeros((16, BS), np.float32)
    b2[:8, 0] = 1.0
    b2[8:, 1] = 1.0
    # partition layout: p = r*16 + e*2 + c  (r = source core, e = expert,
    # c = 512-token half of the core's shard)
    p = np.arange(128)
    pb = p // 64  # batch  (r//4)
    pe = (p % 16) // 2  # expert
    ptok = p // 16 * 2 + p % 2  # token-chunk id (r*2 + c)
    rsmat = ((pb[:, None] == pb[None, :]) & (pe[:, None] == pe[None, :])).astype(
        np.float32
    )
    dsmat = (ptok[:, None] == ptok[None, :]).astype(np.float32)
    ident = np.eye(128, dtype=np.float32)
    ones128 = np.ones((1, 128), np.float32)
    j = np.arange(16)[:, None]
    tt = np.arange(512)[None, :]
    iota_tid = (j * 512 + tt).astype(np.uint16)
    # wrapped dispatch indices for the shared shard: i -> [i%16, i//16], x8 replicas
    ii = np.arange(TSH)
    iw = np.zeros((16, TSH // 16), np.int16)
    iw[ii % 16, ii // 16] = ii

    import ml_dtypes

    bf16 = ml_dtypes.bfloat16
    hs_b = hs_flat.astype(bf16)
    W1_b = W1.astype(bf16)
    W2_b = W2.astype(bf16)
    Ws1_b = np.ascontiguousarray(Ws1.astype(bf16))
    Ws2_b = np.ascontiguousarray(Ws2.astype(bf16))
    in_maps = []
    for c in range(NCORES):
        # extract my expert's 16 rows in (b-major, chunk) order:
        # j = r*2 + cc  ->  partition (j//2)*16 + c*2 + (j%2)
        esel = np.zeros((128, 16), np.float32)
        for j in range(16):
            esel[(j // 2) * 16 + c * 2 + (j % 2), j] = 1.0
        bsel = np.zeros((BS, 1), np.float32)
        bsel[c // 4, 0] = 1.0
        in_maps.append(
            {
                "hs_b": hs_b,
                "hs_sh_b": np.ascontiguousarray(hs_b[c * TSH : (c + 1) * TSH]),
                "hsu_sh": np.ascontiguousarray(hsu_flat[c * TSH : (c + 1) * TSH]),
                "ts": ts,
                "wg": Wg,
                "w1_b": np.ascontiguousarray(W1_b[c]),
                "w2_b": np.ascontiguousarray(W2_b[c]),
                "ws1_b": Ws1_b,
                "ws2_b": Ws2_b,
                "esel": esel,
                "bsel": bsel,
                "b2": b2,
                "lt16": lt16,
                "rsmat": rsmat,
                "dsmat": dsmat,
                "ident": ident,
                "ones128": ones128,
                "iota_tid": iota_tid,
                "iota_sh": np.tile(iw, (8, 1)),
            }
        )
    return in_maps


def combine(results):
    out = np.empty((T, DIM), np.float32)
    for c in range(NCORES):
        out[c * TSH : (c + 1) * TSH] = results[c]["out_shared"].T
    for c in range(NCORES):
        idx = results[c]["out_idx"].reshape(SLOTS)
        out[idx] += results[c]["out_routed"].T
    return out.reshape(BS, SLEN, DIM)


def kernel(**inputs):
    nc = _get_nc()
    in_maps = make_in_maps(inputs)
    res = run_bass_kernel_spmd(nc, in_maps, list(range(NCORES))).results
    return combine(res)


if __name__ == "__main__":
    nc = build_nc()
    print("build ok:", len(nc.inst_map), "instructions")


# revision 22
# speedup vs baseline: 1.2970x; 1.0152x over previous
"""Expert-choice MoE layer (NucleusMoELayer) on 8 Trainium2 NeuronCores.

Strategy (expert-parallel):
 - one expert per core; router + gate-normalization replicated from an
   AllGathered logit table; shared expert sharded over tokens (1024/core)
 - router logits for the core's own 1024-token shard via fp32 PE matmul,
   AllGather -> every core sees all [8 experts x 8192 tokens] logits
 - expert-choice top-1024-per-(batch,expert) via 32-step threshold bisection
   on logits (sigmoid is monotone so logit ranking == score ranking)
 - compaction (selected tokens -> dense slots) via per-partition cumsum +
   GpSimd local_scatter of (token-id, gate-hi, gate-lo) uint16 payloads
 - dispatch: indirect-DMA row gather of selected tokens, PE-transpose to
   feature-major, swiglu FFN in bf16 (weights streamed + cast on the fly)
 - outputs are feature-major; host transposes, concatenates shared shards and
   scatter-adds routed outputs (unique indices per core)

kernel(**inputs) takes FULL unsharded inputs, returns the FULL output.
"""

import sys

if "/opt/trn_rl_repo" not in sys.path:
    sys.path.insert(0, "/opt/trn_rl_repo")

import numpy as np

import concourse.bacc as bacc
import concourse.bass as bass
import concourse.mybir as mybir
import concourse.tile as tile
from concourse.bass_utils import run_bass_kernel_spmd

dt = mybir.dt
AF = mybir.ActivationFunctionType
ALU = mybir.AluOpType

NCORES = 8
BS, SLEN, DIM = 2, 4096, 1024
INNER = 2048
I2 = 2 * INNER  # 4096
E = 8
CAP = 1024  # tokens per (batch, expert)
T = BS * SLEN  # 8192 global tokens
TSH = T // NCORES  # 1024-token shard per core
SLOTS = BS * CAP  # 2048 routed slots per expert
KD = DIM // 128  # 8 k-chunks over dim
MI = I2 // 128  # 32 m-chunks over 2*inner
KI = INNER // 128  # 16 k-chunks over inner
BISECT_ITERS = 32


def build_nc():
    nc = bacc.Bacc(None, target_bir_lowering=False, num_devices=NCORES)

    tens = {}

    def din(name, shape, dtype=dt.float32):
        tens[name] = nc.dram_tensor(name, shape, dtype, kind="ExternalInput")

    def dout(name, shape, dtype=dt.float32):
        tens[name] = nc.dram_tensor(name, shape, dtype, kind="ExternalOutput")

    din("hs_b", [T, DIM], dt.bfloat16)
    din("hs_sh_b", [TSH, DIM], dt.bfloat16)
    din("hsu_sh", [TSH, DIM])
    din("ts", [BS, DIM])
    din("wg", [E, 2 * DIM])
    din("w1_b", [DIM, I2], dt.bfloat16)
    din("w2_b", [INNER, DIM], dt.bfloat16)
    din("ws1_b", [DIM, I2], dt.bfloat16)
    din("ws2_b", [INNER, DIM], dt.bfloat16)
    din("esel", [128, 16])
    din("bsel", [BS, 1])
    din("b2", [16, BS])
    din("lt16", [16, 16])
    din("rsmat", [128, 128])
    din("dsmat", [128, 128])
    din("ident", [128, 128])
    din("ones128", [1, 128])
    din("iota_tid", [16, 512], dt.uint16)
    din("iota_sh", [128, 64], dt.int16)
    dout("out_routed", [DIM, SLOTS])
    dout("out_idx", [SLOTS, 1], dt.int32)
    dout("out_shared", [DIM, TSH])
    dout("dbg_thr", [128, 1])
    dout("dbg_gate", [BS, CAP])

    with tile.TileContext(nc, num_cores=NCORES) as tc:
        _emit(nc, tc, tens)
    nc.finalize()
    return nc


def _load_weights_bf16(nc, pools, wdram, nk, width, tagp):
    """Load [nk*128, width] bf16 DRAM weights into resident k-tiles."""
    wr_pool = pools["wres"]
    tiles = []
    for k in range(nk):
        wt = wr_pool.tile([128, width], dt.bfloat16, tag=f"{tagp}{k}")
        nc.sync.dma_start(wt[:], wdram[k * 128 : (k + 1) * 128, :])
        tiles.append(wt)
    return tiles


def _ffn_chunk(nc, pools, wr1, wr2, xT, rhs_off, out_dram, out_col, gate_sb, hb=0):
    """One 512-token swiglu FFN chunk, feature-major, resident bf16 weights.
    wr1: 8 k-tiles [128, I2]; wr2: 16 k-tiles [128, DIM]; xT [128, KD*xw]."""
    sb, sb1, pmm = pools["sb"], pools["sb1"], pools["pmm"]
    xw = xT.shape[1] // KD
    h_sb = sb1.tile([128, 16 * 512], dt.bfloat16, tag=f"h_sb{hb}")
    for i in range(16):
        ps_a = pmm.tile([128, 512], dt.float32, tag="mm")
        for k in range(KD):
            nc.tensor.matmul(
                ps_a[:],
                lhsT=wr1[k][:, i * 128 : (i + 1) * 128],
                rhs=xT[:, k * xw + rhs_off : k * xw + rhs_off + 512],
                start=(k == 0),
                stop=(k == KD - 1),
            )
        ps_g = pmm.tile([128, 512], dt.float32, tag="mm")
        for k in range(KD):
            nc.tensor.matmul(
                ps_g[:],
                lhsT=wr1[k][:, (16 + i) * 128 : (17 + i) * 128],
                rhs=xT[:, k * xw + rhs_off : k * xw + rhs_off + 512],
                start=(k == 0),
                stop=(k == KD - 1),
            )
        sl = sb.tile([128, 512], dt.bfloat16, tag="silu")
        nc.scalar.activation(sl[:], ps_g[:], AF.Silu)
        nc.vector.tensor_mul(h_sb[:, i * 512 : (i + 1) * 512], ps_a[:], sl[:])
    for mo in range(KD):
        ps2 = pmm.tile([128, 512], dt.float32, tag="mm")
        for k2 in range(KI):
            nc.tensor.matmul(
                ps2[:],
                lhsT=wr2[k2][:, mo * 128 : (mo + 1) * 128],
                rhs=h_sb[:, k2 * 512 : (k2 + 1) * 512],
                start=(k2 == 0),
                stop=(k2 == KI - 1),
            )
        yo = sb.tile([128, 512], dt.float32, tag="yo")
        if gate_sb is not None:
            nc.vector.tensor_mul(yo[:], ps2[:], gate_sb[:])
        else:
            nc.scalar.activation(yo[:], ps2[:], AF.Copy)
        nc.sync.dma_start(
            out_dram[mo * 128 : (mo + 1) * 128, out_col : out_col + 512], yo[:]
        )


def _emit(nc, tc, t):
    from contextlib import ExitStack

    ctx = ExitStack()
    with ctx:
        const = ctx.enter_context(tc.tile_pool(name="const", bufs=1))
        sb = ctx.enter_context(tc.tile_pool(name="sb", bufs=2))
        sb1 = ctx.enter_context(tc.tile_pool(name="sb1", bufs=1))
        wres = ctx.enter_context(tc.tile_pool(name="wres", bufs=1))
        cw = ctx.enter_context(tc.tile_pool(name="cw", bufs=1))  # [16,*] scratch
        bis = ctx.enter_context(tc.tile_pool(name="bis", bufs=1))
        dr = ctx.enter_context(tc.tile_pool(name="dr", bufs=1, space="DRAM"))
        pmm = ctx.enter_context(tc.tile_pool(name="pmm", bufs=4, space="PSUM"))
        ptr = ctx.enter_context(tc.tile_pool(name="ptr", bufs=2, space="PSUM"))
        psm = ctx.enter_context(tc.tile_pool(name="psm", bufs=2, space="PSUM"))
        pools = {"sb": sb, "sb1": sb1, "wres": wres, "pmm": pmm}

        def load_const(name, shape, dtype=dt.float32):
            tl = const.tile(shape, dtype, tag=name)
            nc.sync.dma_start(tl[:], t[name][:])
            return tl

        esel = load_const("esel", [128, 16])
        bsel = load_const("bsel", [BS, 1])
        b2 = load_const("b2", [16, BS])
        lt16 = load_const("lt16", [16, 16])
        rsmat = load_const("rsmat", [128, 128])
        dsmat = load_const("dsmat", [128, 128])
        ident = load_const("ident", [128, 128])
        ones128 = load_const("ones128", [1, 128])
        iota_tid = load_const("iota_tid", [16, 512], dt.uint16)
        iota_sh = load_const("iota_sh", [128, 64], dt.int16)

        # ================= router =================
        wg_sb = sb1.tile([E, 2 * DIM], dt.float32, tag="h_sb0")
        nc.sync.dma_start(wg_sb[:], t["wg"][:])
        wgT = sb1.tile([128, 16 * E], dt.float32, tag="wgT")
        for k in range(16):
            ps = ptr.tile([128, 128], dt.float32, tag="tr")
            nc.tensor.transpose(
                out=ps[:, :E],
                in_=wg_sb[:, k * 128 : (k + 1) * 128],
                identity=ident[:E, :E],
            )
            nc.vector.tensor_copy(wgT[:, k * E : (k + 1) * E], ps[:, :E])

        ts_sb = cw.tile([BS, DIM], dt.float32, tag="cwf")
        nc.sync.dma_start(ts_sb[:], t["ts"][:])
        tsT = sb1.tile([128, KD * BS], dt.float32, tag="tsT")
        for k in range(KD):
            ps = ptr.tile([128, 128], dt.float32, tag="tr")
            nc.tensor.transpose(
                out=ps[:, :BS],
                in_=ts_sb[:, k * 128 : (k + 1) * 128],
                identity=ident[:BS, :BS],
            )
            nc.vector.tensor_copy(tsT[:, k * BS : (k + 1) * BS], ps[:, :BS])

        biasT_ps = psm.tile([BS, E], dt.float32, tag="small")
        for k in range(KD):
            nc.tensor.matmul(
                biasT_ps[:],
                lhsT=tsT[:, k * BS : (k + 1) * BS],
                rhs=wgT[:, k * E : (k + 1) * E],
                start=(k == 0),
                stop=(k == KD - 1),
            )
        biasT_sb = sb1.tile([BS, E], dt.float32, tag="biasT")
        nc.vector.tensor_copy(biasT_sb[:], biasT_ps[:])
        bias_ps = psm.tile([E, 1], dt.float32, tag="small")
        nc.tensor.matmul(bias_ps[:], lhsT=biasT_sb[:], rhs=bsel[:], start=True, stop=True)
        bias_mine = sb1.tile([E, 1], dt.float32, tag="bias_mine")
        nc.vector.tensor_copy(bias_mine[:], bias_ps[:])

        ag_in = dr.tile([E, TSH], dt.float32)
        for n in range(TSH // 512):
            hsuT = sb1.tile([128, KD * 512], dt.float32, tag="h_sb1")
            for tt in range(4):
                row = sb.tile([128, DIM], dt.float32, tag="row_in")
                nc.sync.dma_start(
                    row[:], t["hsu_sh"][n * 512 + tt * 128 : n * 512 + (tt + 1) * 128, :]
                )
                for k in range(KD):
                    ps = ptr.tile([128, 128], dt.float32, tag="tr")
                    nc.tensor.transpose(
                        out=ps[:], in_=row[:, k * 128 : (k + 1) * 128], identity=ident[:]
                    )
                    nc.vector.tensor_copy(
                        hsuT[:, k * 512 + tt * 128 : k * 512 + (tt + 1) * 128], ps[:]
                    )
            lps = pmm.tile([128, 512], dt.float32, tag="mm")
            for k in range(KD):
                nc.tensor.matmul(
                    lps[:E, :],
                    lhsT=wgT[:, (8 + k) * E : (9 + k) * E],
                    rhs=hsuT[:, k * 512 : (k + 1) * 512],
                    start=(k == 0),
                    stop=(k == KD - 1),
                )
            lchunk = sb.tile([E, 512], dt.float32, tag="yo")
            nc.vector.tensor_scalar(
                lchunk[:], lps[:E, :], bias_mine[:], None, op0=ALU.add
            )
            nc.sync.dma_start(ag_in[:, n * 512 : (n + 1) * 512], lchunk[:])

        wrs1 = _load_weights_bf16(nc, pools, t["ws1_b"], KD, I2, "wr1_")
        wrs2 = _load_weights_bf16(nc, pools, t["ws2_b"], KI, DIM, "wr2_")

        ag_out = dr.tile([NCORES * E, TSH], dt.float32, addr_space="Shared")
        nc.gpsimd.collective_compute(
            "AllGather",
            ALU.bypass,
            replica_groups=[list(range(NCORES))],
            ins=[ag_in[:]],
            outs=[ag_out[:]],
        )
        logit_all = sb1.tile([128, 512], dt.float32, tag="logit_all")
        nc.sync.dma_start(
            logit_all[:], ag_out[:].rearrange("(r e) (c t) -> (r e c) t", e=E, c=2)
        )

        # ============ shared expert (PE filler during bisection) ============
        for n in range(TSH // 512):
            xsT = sb1.tile([128, KD * 512], dt.bfloat16, tag="xT")
            nc.gpsimd.dma_gather(
                out_ap=xsT[:].rearrange("p (k t) -> p k t", t=512),
                in_ap=t["hs_sh_b"][:],
                idxs_ap=iota_sh[:, n * 32 : (n + 1) * 32],
                num_idxs=512,
                num_idxs_reg=512,
                elem_size=DIM,
                transpose=True,
            )
            _ffn_chunk(
                nc, pools, wrs1, wrs2, xsT, 0, t["out_shared"], n * 512, None,
                hb=n % 2,
            )

        # ================= bisection =================
        lo = sb1.tile([128, 1], dt.float32, tag="lo")
        hi = sb1.tile([128, 1], dt.float32, tag="hi")
        nc.vector.memset(lo[:], -16.0)
        nc.vector.memset(hi[:], 16.0)
        for _ in range(BISECT_ITERS):
            mid = bis.tile([128, 1], dt.float32, tag="mid")
            nc.vector.tensor_add(mid[:], lo[:], hi[:])
            nc.vector.tensor_scalar_mul(mid[:], mid[:], 0.5)
            cmp = bis.tile([128, 512], dt.float32, tag="cmp")
            nc.vector.tensor_scalar(cmp[:], logit_all[:], mid[:], None, op0=ALU.is_ge)
            cnt = bis.tile([128, 1], dt.float32, tag="cnt")
            nc.vector.tensor_reduce(cnt[:], cmp[:], mybir.AxisListType.X, ALU.add)
            cntg_ps = psm.tile([128, 1], dt.float32, tag="small")
            nc.tensor.matmul(
                cntg_ps[:], lhsT=rsmat[:], rhs=cnt[:], start=True, stop=True
            )
            cntg = bis.tile([128, 1], dt.float32, tag="cntg")
            nc.vector.tensor_copy(cntg[:], cntg_ps[:])
            pred = bis.tile([128, 1], dt.uint8, tag="pred")
            nc.vector.tensor_scalar(pred[:], cntg[:], float(CAP), None, op0=ALU.is_ge)
            nc.vector.copy_predicated(lo[:], pred[:], mid[:])
            nc.vector.tensor_scalar(pred[:], cntg[:], float(CAP), None, op0=ALU.is_lt)
            nc.vector.copy_predicated(hi[:], pred[:], mid[:])
        nc.sync.dma_start(t["dbg_thr"][:], lo[:])

        # ================= gates =================
        mask = sb1.tile([128, 512], dt.float32, tag="mask")
        nc.vector.tensor_scalar(mask[:], logit_all[:], lo[:], None, op0=ALU.is_ge)
        sig = sb1.tile([128, 512], dt.float32, tag="sig")
        nc.scalar.activation(sig[:], logit_all[:], AF.Sigmoid)
        g = sb1.tile([128, 512], dt.float32, tag="g")
        nc.vector.tensor_mul(g[:], sig[:], mask[:])
        d_ps = pmm.tile([128, 512], dt.float32, tag="mm")
        nc.tensor.matmul(d_ps[:], lhsT=dsmat[:], rhs=g[:], start=True, stop=True)
        dsafe = cw.tile([16 * 8, 512], dt.float32, tag="cwb")
        nc.vector.tensor_scalar(dsafe[:], d_ps[:], 1e-12, None, op0=ALU.add)
        drec = cw.tile([16 * 8, 512], dt.float32, tag="cwa")
        nc.vector.reciprocal(drec[:], dsafe[:])
        ghat = sb1.tile([128, 512], dt.float32, tag="ghat")
        nc.vector.tensor_mul(ghat[:], g[:], drec[:])

        ghm_ps = psm.tile([16, 512], dt.float32, tag="small")
        nc.tensor.matmul(ghm_ps[:], lhsT=esel[:], rhs=ghat[:], start=True, stop=True)
        ghat_mine = sb1.tile([16, 512], dt.float32, tag="ghat_mine")
        nc.vector.tensor_copy(ghat_mine[:], ghm_ps[:])
        msk_ps = psm.tile([16, 512], dt.float32, tag="small")
        nc.tensor.matmul(msk_ps[:], lhsT=esel[:], rhs=mask[:], start=True, stop=True)
        mask_mine = sb1.tile([16, 512], dt.float32, tag="mask_mine")
        nc.vector.tensor_copy(mask_mine[:], msk_ps[:])

        # ================= compaction =================
        incl = sb1.tile([16, 512], dt.float32, tag="incl")
        nc.vector.tensor_tensor_scan(
            incl[:], mask_mine[:], mask_mine[:], 0.0, op0=ALU.add, op1=ALU.bypass
        )
        offs_ps = psm.tile([16, 1], dt.float32, tag="small")
        nc.tensor.matmul(
            offs_ps[:], lhsT=lt16[:], rhs=incl[:, 511:512], start=True, stop=True
        )
        # pos (within-b slot) = (incl - mask) + offs - b*CAP
        pos = sb1.tile([16, 512], dt.float32, tag="pos")
        nc.vector.tensor_sub(pos[:], incl[:], mask_mine[:])
        offs = cw.tile([16, 1], dt.float32, tag="cwo")
        nc.vector.tensor_copy(offs[:], offs_ps[:])
        nc.vector.tensor_scalar(pos[:], pos[:], offs[:], None, op0=ALU.add)
        boff = cw.tile([16, 1], dt.float32, tag="cwo")
        nc.vector.tensor_scalar(boff[:], b2[:, 1:2], float(CAP), None, op0=ALU.mult)
        nc.vector.tensor_scalar(pos[:], pos[:], boff[:], None, op0=ALU.subtract)
        # invalid/overflow -> -1:  p1 = (pos+1) * ok - 1
        okm = cw.tile([16, 512], dt.float32, tag="cwa")
        nc.vector.tensor_scalar(okm[:], pos[:], float(CAP - 1), None, op0=ALU.is_le)
        nc.vector.tensor_mul(okm[:], okm[:], mask_mine[:])
        p1 = cw.tile([16, 512], dt.float32, tag="cwb")
        nc.vector.tensor_scalar(p1[:], pos[:], 1.0, None, op0=ALU.add)
        nc.vector.tensor_mul(p1[:], p1[:], okm[:])
        nc.vector.tensor_scalar(p1[:], p1[:], 1.0, None, op0=ALU.subtract)
        pos_i16 = sb1.tile([16, 512], dt.int16, tag="pos_i16")
        nc.vector.tensor_copy(pos_i16[:], p1[:])

        gbits = ghat_mine[:].bitcast(dt.uint16).rearrange("p (t two) -> p t two", two=2)
        glo = sb1.tile([16, 512], dt.uint16, tag="glo")
        nc.vector.tensor_copy(glo[:, :, None], gbits[:, :, 0:1])
        ghi = sb1.tile([16, 512], dt.uint16, tag="ghi")
        nc.vector.tensor_copy(ghi[:, :, None], gbits[:, :, 1:2])

        cc = {}
        for name, data in (("tid", iota_tid), ("ghi", ghi), ("glo", glo)):
            so = cw.tile([16, CAP], dt.uint16, tag="cws")
            nc.gpsimd.local_scatter(
                out_ap=so[:],
                data_ap=data[:],
                idxs_ap=pos_i16[:],
                channels=16,
                num_elems=CAP,
                num_idxs=512,
            )
            sf = cw.tile([16, CAP], dt.float32, tag="cwf")
            nc.vector.tensor_copy(sf[:], so[:])
            # collapse 16 partitions -> [2, CAP] (one row per batch)
            ccn = sb1.tile([BS, CAP], dt.float32, tag=f"cc_{name}")
            for h in range(2):
                cps = psm.tile([BS, 512], dt.float32, tag="small")
                nc.tensor.matmul(
                    cps[:],
                    lhsT=b2[:],
                    rhs=sf[:, h * 512 : (h + 1) * 512],
                    start=True,
                    stop=True,
                )
                nc.vector.tensor_copy(ccn[:, h * 512 : (h + 1) * 512], cps[:])
            cc[name] = ccn

        # gates: f32 bits = ghi*65536 + glo (int32, exact)
        c64k = cw.tile([BS, 1], dt.int32, tag="c64k")
        nc.vector.memset(c64k[:], 65536)
        glo_i = cw.tile([BS, CAP], dt.int32, tag="cwh")
        nc.vector.tensor_copy(glo_i[:], cc["glo"][:])
        gbits_i = sb1.tile([BS, CAP], dt.int32, tag="gbits_i")
        nc.vector.tensor_copy(gbits_i[:], cc["ghi"][:])
        nc.vector.tensor_tensor(
            gbits_i[:], gbits_i[:], c64k[:].to_broadcast([BS, CAP]), op=ALU.mult
        )
        nc.vector.tensor_add(gbits_i[:], gbits_i[:], glo_i[:])
        gatec = gbits_i[:].bitcast(dt.float32)
        nc.sync.dma_start(t["dbg_gate"][:], gatec)
        gate_buf = dr.tile([BS, CAP], dt.float32)
        nc.sync.dma_start(gate_buf[:], gatec)

        tid_i = cw.tile([BS, CAP], dt.int32, tag="cwh")
        nc.vector.tensor_copy(tid_i[:], cc["tid"][:])
        nc.sync.dma_start(
            t["out_idx"][:].rearrange("(b t) one -> b (t one)", b=BS), tid_i[:]
        )
        idx_buf = dr.tile([SLOTS, 1], dt.int32)
        nc.sync.dma_start(
            idx_buf[:].rearrange("(b t) one -> b (t one)", b=BS), tid_i[:]
        )

        # ================= dispatch + expert FFN =================
        wr1 = _load_weights_bf16(nc, pools, t["w1_b"], KD, I2, "wr1_")
        wr2 = _load_weights_bf16(nc, pools, t["w2_b"], KI, DIM, "wr2_")
        # int16 slot->token table to DRAM for the gather engine
        tid_i16 = cw.tile([BS, CAP], dt.int16, tag="cws")
        nc.vector.tensor_copy(tid_i16[:], cc["tid"][:])
        idx16_buf = dr.tile([SLOTS, 1], dt.int16)
        nc.sync.dma_start(
            idx16_buf[:].rearrange("(b t) one -> b (t one)", b=BS), tid_i16[:]
        )
        # wrapped view: chunk n, slot i -> [i%16, n*32 + i//16]
        idx16_w = idx16_buf[:].rearrange("(n c p) one -> p (n c one)", p=16, c=32)
        for n in range(SLOTS // 512):
            idxw = sb.tile([128, 32], dt.int16, tag="idxw")
            for rep in range(8):
                nc.sync.dma_start(
                    idxw[rep * 16 : (rep + 1) * 16, :],
                    idx16_w[:, n * 32 : (n + 1) * 32],
                )
            xT = sb1.tile([128, KD * 512], dt.bfloat16, tag="xT")
            nc.gpsimd.dma_gather(
                out_ap=xT[:].rearrange("p (k t) -> p k t", t=512),
                in_ap=t["hs_b"][:],
                idxs_ap=idxw[:],
                num_idxs=512,
                num_idxs_reg=512,
                elem_size=DIM,
                transpose=True,
            )
            grow = sb1.tile([1, 512], dt.float32, tag="sig")
            nc.sync.dma_start(
                grow[:],
                gate_buf[:].rearrange("b (m t) -> (b m) t", t=512)[n : n + 1, :],
            )
            grep_ps = pmm.tile([128, 512], dt.float32, tag="mm")
            nc.tensor.matmul(
                grep_ps[:], lhsT=ones128[:], rhs=grow[:], start=True, stop=True
            )
            gate_sb = sb1.tile([128, 512], dt.float32, tag="mask")
            nc.vector.tensor_copy(gate_sb[:], grep_ps[:])
            _ffn_chunk(
                nc, pools, wr1, wr2, xT, 0, t["out_routed"], n * 512, gate_sb,
                hb=n % 2,
            )


# ======================= host side =======================

_CACHED_NC = None


def _get_nc():
    global _CACHED_NC
    if _CACHED_NC is None:
        _CACHED_NC = build_nc()
    return _CACHED_NC


def make_in_maps(inputs):
    hs_flat = np.ascontiguousarray(
        np.asarray(inputs["hidden_states"], dtype=np.float32).reshape(T, DIM)
    )
    hsu_flat = np.ascontiguousarray(
        np.asarray(inputs["hidden_states_unmodulated"], dtype=np.float32).reshape(
            T, DIM
        )
    )
    ts = np.asarray(inputs["timestep"], dtype=np.float32)
    Wg = np.asarray(inputs["Wg"], dtype=np.float32)
    W1 = np.asarray(inputs["W1"], dtype=np.float32)
    W2 = np.asarray(inputs["W2"], dtype=np.float32)
    Ws1 = np.ascontiguousarray(np.asarray(inputs["Ws1"], dtype=np.float32))
    Ws2 = np.ascontiguousarray(np.asarray(inputs["Ws2"], dtype=np.float32))

    lt16 = np.triu(np.ones((16, 16), np.float32), 1)  # lhsT[k,m]=1 iff k<m
    b2 = np.zeros((16, BS), np.float32)
    b2[:8, 0] = 1.0
    b2[8:, 1] = 1.0
    # partition layout: p = r*16 + e*2 + c  (r = source core, e = expert,
    # c = 512-token half of the core's shard)
    p = np.arange(128)
    pb = p // 64  # batch  (r//4)
    pe = (p % 16) // 2  # expert
    ptok = p // 16 * 2 + p % 2  # token-chunk id (r*2 + c)
    rsmat = ((pb[:, None] == pb[None, :]) & (pe[:, None] == pe[None, :])).astype(
        np.float32
    )
    dsmat = (ptok[:, None] == ptok[None, :]).astype(np.float32)
    ident = np.eye(128, dtype=np.float32)
    ones128 = np.ones((1, 128), np.float32)
    j = np.arange(16)[:, None]
    tt = np.arange(512)[None, :]
    iota_tid = (j * 512 + tt).astype(np.uint16)
    # wrapped dispatch indices for the shared shard: i -> [i%16, i//16], x8 replicas
    ii = np.arange(TSH)
    iw = np.zeros((16, TSH // 16), np.int16)
    iw[ii % 16, ii // 16] = ii

    import ml_dtypes

    bf16 = ml_dtypes.bfloat16
    hs_b = hs_flat.astype(bf16)
    W1_b = W1.astype(bf16)
    W2_b = W2.astype(bf16)
    Ws1_b = np.ascontiguousarray(Ws1.astype(bf16))
    Ws2_b = np.ascontiguousarray(Ws2.astype(bf16))
    in_maps = []
    for c in range(NCORES):
        # extract my expert's 16 rows in (b-major, chunk) order:
        # j = r*2 + cc  ->  partition (j//2)*16 + c*2 + (j%2)
        esel = np.zeros((128, 16), np.float32)
        for j in range(16):
            esel[(j // 2) * 16 + c * 2 + (j % 2), j] = 1.0
        bsel = np.zeros((BS, 1), np.float32)
        bsel[c // 4, 0] = 1.0
        in_maps.append(
            {
                "hs_b": hs_b,
                "hs_sh_b": np.ascontiguousarray(hs_b[c * TSH : (c + 1) * TSH]),
                "hsu_sh": np.ascontiguousarray(hsu_flat[c * TSH : (c + 1) * TSH]),
                "ts": ts,
                "wg": Wg,
                "w1_b": np.ascontiguousarray(W1_b[c]),
                "w2_b": np.ascontiguousarray(W2_b[c]),
                "ws1_b": Ws1_b,
                "ws2_b": Ws2_b,
                "esel": esel,
                "bsel": bsel,
                "b2": b2,
                "lt16": lt16,
                "rsmat": rsmat,
                "dsmat": dsmat,
                "ident": ident,
                "ones128": ones128,
                "iota_tid": iota_tid,
                "iota_sh": np.tile(iw, (8, 1)),
            }
        )
    return in_maps


def combine(results):
    out = np.empty((T, DIM), np.float32)
    for c in range(NCORES):
        out[c * TSH : (c + 1) * TSH] = results[c]["out_shared"].T
    for c in range(NCORES):
        idx = results[c]["out_idx"].reshape(SLOTS)
        out[idx] += results[c]["out_routed"].T
    return out.reshape(BS, SLEN, DIM)


def kernel(**inputs):
    nc = _get_nc()
    in_maps = make_in_maps(inputs)
    res = run_bass_kernel_spmd(nc, in_maps, list(range(NCORES))).results
    return combine(res)


if __name__ == "__main__":
    nc = build_nc()
    print("build ok:", len(nc.inst_map), "instructions")
